# revision 2
# baseline (speedup 1.0000x reference)
"""ConvLocalAttention2d (7x7 window, 4 heads, d_head=16) on 8 trn2 NeuronCores.

Sharding: one (batch, head) pair per core  (B=2 x heads=4 = 8 cores), zero
cross-core communication.

Per-core algorithm (d=16, H=W=96, K=7x7=49):
  - Queries are processed in 8x16 = 128-pixel tiles (12 x 6 = 72 tiles).
  - For each tile, the key/value halo is 14x22 = 308 pixels, split into key
    chunks of (128, 128, 52) along the flattened halo.
  - C1 (scores, transposed): per chunk, PE matmul
        scoresT[keys, 128q] = k_aug_chunk.T @ q_aug_tile
    contracting over 52 channels: 16 data + 14 y-onehot (mod 14) + 22
    x-onehot (mod 22).  Position channels add 0 for in-window pairs, -1e9
    for out-of-window pairs (exact: mod-14/mod-22 never alias within the
    +-10 / +-18 offset range of a tile).  128-col chunks keep LDWEIGHTS on
    the fast-weight-load path.
  - exp: one ACT pass per 4 tiles, PSUM->SBUF, scale=1/sqrt(16), out bf16.
  - C2: PE matmul out_u[128q, 17] += expT_chunk.T @ vt_chunk; vt column 16
    is an inside-image indicator whose output is the softmax denominator Z.
  - One DVE copy per 4 tiles moves out_u PSUM->SBUF; the final divide by Z
    happens on the host (it is not part of the memory-bound device loop).
"""

import functools
import numpy as np
import ml_dtypes

B = 2
HEADS = 4
DH = 16
H = W = 96
PAD = 3
PH = PW = H + 2 * PAD  # 102
TY, TX = 8, 16           # query tile shape
GY, GX = H // TY, W // TX  # 12 x 6 tile grid
NT = GY * GX             # 72 tiles
HY, HX = TY + 6, TX + 6  # halo 14 x 22 = 308 keys
NK = HY * HX
MY, MX = 14, 22          # mask moduli (no aliasing for |dy|<=10, |dx|<=18)
D = DH + MY + MX         # 52 contraction channels
NEG = -1.0e9
CHUNKS = [(0, 128), (128, 128), (256, 52)]  # (key0, nkeys)
NC = len(CHUNKS)
GB = 4                   # tiles per exp/PSUM group
NG = NT // GB            # 18 groups
LGRP = 12                # tiles per load/store DMA slice

BF16 = ml_dtypes.bfloat16

Y_OK = {d % MY for d in range(-3, 4)}
X_OK = {d % MX for d in range(-3, 4)}


@functools.lru_cache(maxsize=1)
def _mask_channels():
    """Constant (core-independent) aug channels.

    q side: [MY+MX, H, W] one-hots of (y mod 14) and (x mod 22).
    k side: [MY+MX, PH, PW] with 0 where the channel does not veto the
    pair, NEG where it does.
    """
    qm = np.zeros((MY + MX, H, W), np.float32)
    yy = np.arange(H)
    xx = np.arange(W)
    for r in range(MY):
        qm[r, yy % MY == r, :] = 1.0
    for r in range(MX):
        qm[MY + r, :, xx % MX == r] = 1.0

    km = np.zeros((MY + MX, PH, PW), np.float32)
    py = np.arange(PH)
    px = np.arange(PW)
    for r in range(MY):
        bad = np.array([((y - PAD - r) % MY) not in Y_OK for y in py])
        km[r, bad, :] = NEG
    for r in range(MX):
        bad = np.array([((x - PAD - r) % MX) not in X_OK for x in px])
        km[MY + r, :, bad] = NEG
    return qm, km


def _host_prep(q, k, v):
    """Full [2,64,96,96] f32 inputs -> list of 8 per-core input dicts."""
    qm, km = _mask_channels()
    in_maps = []
    for core in range(8):
        b, h = divmod(core, HEADS)
        qs = q[b, DH * h:DH * h + DH]          # [16, 96, 96]
        ks = k[b, DH * h:DH * h + DH]
        vs = v[b, DH * h:DH * h + DH]

        q_aug = np.empty((D, H, W), np.float32)
        q_aug[:DH] = qs
        q_aug[DH:] = qm
        # [D, GY, TY, GX, TX] -> [D, GY*GX, TY*TX]
        q_tiled = np.ascontiguousarray(
            q_aug.reshape(D, GY, TY, GX, TX).transpose(0, 1, 3, 2, 4)
            .reshape(D, NT, TY * TX))

        k_aug = np.empty((D, PH, PW), np.float32)
        k_aug[:DH] = 0.0
        k_aug[:DH, PAD:PAD + H, PAD:PAD + W] = ks
        k_aug[DH:] = km
        # per-tile halo, flattened: [D, 72, 308]
        k_tiled = np.empty((D, NT, NK), np.float32)
        for ty in range(GY):
            for tx in range(GX):
                k_tiled[:, ty * GX + tx] = k_aug[
                    :, TY * ty:TY * ty + HY,
                    TX * tx:TX * tx + HX].reshape(D, NK)

        vt = np.zeros((PH, PW, DH + 1), np.float32)
        vt[PAD:PAD + H, PAD:PAD + W, :DH] = np.transpose(vs, (1, 2, 0))
        vt[PAD:PAD + H, PAD:PAD + W, DH] = 1.0
        # key-within-chunk major: [128, 72, 3, 17]
        vt_res = np.zeros((128, NT, NC, DH + 1), np.float32)
        for ty in range(GY):
            for tx in range(GX):
                halo = vt[TY * ty:TY * ty + HY,
                          TX * tx:TX * tx + HX, :].reshape(NK, DH + 1)
                for c, (k0, nk) in enumerate(CHUNKS):
                    vt_res[:nk, ty * GX + tx, c, :] = halo[k0:k0 + nk]

        in_maps.append({
            "q_tiled": q_tiled.astype(BF16),
            "k_tiled": k_tiled.astype(BF16),
            "vt_res": vt_res.astype(BF16),
        })
    return in_maps


@functools.lru_cache(maxsize=1)
def _build_program():
    from contextlib import ExitStack
    import concourse.bass as bass
    import concourse.tile as tile
    from concourse import bacc, mybir

    f32 = mybir.dt.float32
    bf16 = mybir.dt.bfloat16

    nc = bacc.Bacc("TRN2", target_bir_lowering=False, debug=False,
                   num_devices=8)
    q_d = nc.dram_tensor("q_tiled", [D, NT, TY * TX], bf16,
                         kind="ExternalInput").ap()
    k_d = nc.dram_tensor("k_tiled", [D, NT, NK], bf16,
                         kind="ExternalInput").ap()
    vt_d = nc.dram_tensor("vt_res", [128, NT, NC, DH + 1], bf16,
                          kind="ExternalInput").ap()
    # un-normalized [query-in-tile, tile, 17]; host divides by column 16
    out_d = nc.dram_tensor("out", [TY * TX, NT, DH + 1], f32,
                           kind="ExternalOutput").ap()

    with tile.TileContext(nc) as tc:
        with ExitStack() as ctx:
            cpool = ctx.enter_context(tc.tile_pool(name="const", bufs=1))
            epool = ctx.enter_context(tc.tile_pool(name="expT", bufs=3))
            pp_s = ctx.enter_context(
                tc.tile_pool(name="ps_scores", bufs=2, space="PSUM"))
            pp_o = ctx.enter_context(
                tc.tile_pool(name="ps_out", bufs=2, space="PSUM"))

            q_sb = cpool.tile([D, NT, TY * TX], bf16)
            k_sb = cpool.tile([D, NT, NK], bf16)
            vt_sb = cpool.tile([128, NT, NC, DH + 1], bf16)
            out_sb = cpool.tile([TY * TX, NT, DH + 1], f32)
            # sliced loads so early tiles unblock before the full load lands
            for g in range(NT // LGRP):
                s = slice(LGRP * g, LGRP * (g + 1))
                nc.sync.dma_start(k_sb[:, s, :], k_d[:, s, :])
                nc.sync.dma_start(q_sb[:, s, :], q_d[:, s, :])
                nc.sync.dma_start(vt_sb[:, s, :, :], vt_d[:, s, :, :])

            exps = [None] * NG
            outs = [None] * NG

            def emit_c1(g):
                scores = pp_s.tile([128, 3 * GB, 128], f32, tag="scores")
                for i in range(GB):
                    t = GB * g + i
                    for c, (k0, nk) in enumerate(CHUNKS):
                        nc.tensor.matmul(
                            scores[:nk, 3 * i + c, :],
                            lhsT=k_sb[:, t, k0:k0 + nk],
                            rhs=q_sb[:, t, :],
                            start=True, stop=True)
                expT = epool.tile([128, 3 * GB, 128], bf16, tag="expT")
                nc.scalar.activation(expT[:], scores[:],
                                     mybir.ActivationFunctionType.Exp,
                                     scale=0.25)
                exps[g] = expT

            def emit_c2(g):
                expT = exps[g]
                out_u = pp_o.tile([128, GB, 32], f32, tag="outu")
                for i in range(GB):
                    t = GB * g + i
                    for c, (k0, nk) in enumerate(CHUNKS):
                        nc.tensor.matmul(
                            out_u[:, i, :DH + 1],
                            lhsT=expT[:nk, 3 * i + c, :],
                            rhs=vt_sb[:nk, t, c, :],
                            start=(c == 0), stop=(c == 2))
                nc.vector.tensor_copy(
                    out_sb[:, GB * g:GB * g + GB, :],
                    out_u[:, :, :DH + 1])
                outs[g] = True
                # store as soon as a 12-tile span is done
                if (g + 1) % (LGRP // GB) == 0:
                    s = slice(GB * g + GB - LGRP, GB * g + GB)
                    nc.sync.dma_start(out_d[:, s, :], out_sb[:, s, :])

            # software pipeline: PE never waits on the exp it just issued
            for g in range(NG):
                emit_c1(g)
                if g >= 2:
                    emit_c2(g - 2)
            emit_c2(NG - 2)
            emit_c2(NG - 1)
    nc.compile()
    return nc


def kernel(q, k, v):
    from concourse.bass_utils import run_bass_kernel_spmd

    nc = _build_program()
    in_maps = _host_prep(np.asarray(q, np.float32), np.asarray(k, np.float32),
                         np.asarray(v, np.float32))
    res = run_bass_kernel_spmd(nc, in_maps, list(range(8)))

    out = np.empty((B, HEADS, DH, H, W), np.float32)
    for core in range(8):
        b, h = divmod(core, HEADS)
        o = res.results[core]["out"]           # [TY*TX, NT, 17]
        o = o[:, :, :DH] / o[:, :, DH:DH + 1]  # softmax denominator
        # [TY*TX, NT, DH] -> [qy,qx,ty,tx,d] -> [ty,qy,tx,qx,d] -> [H,W,DH]
        o = o.reshape(TY, TX, GY, GX, DH)
        o = o.transpose(2, 0, 3, 1, 4).reshape(H, W, DH)
        out[b, h] = np.transpose(o, (2, 0, 1))
    return out.reshape(B, HEADS * DH, H, W)


# revision 3
# speedup vs baseline: 1.2577x; 1.2577x over previous
"""ConvLocalAttention2d (7x7 window, 4 heads, d_head=16) on 8 trn2 NeuronCores.

Sharding: one (batch, head) pair per core  (B=2 x heads=4 = 8 cores), zero
cross-core communication.

Per-core algorithm (d=16, H=W=96, K=7x7=49):
  - Queries are processed in 8x16 = 128-pixel tiles (12 x 6 = 72 tiles).
  - For each tile, the key/value halo is 14x22 = 308 pixels, split into key
    chunks of (128, 115, 65) along the flattened halo.  All chunk sizes and
    the 65-channel contraction stay >= 65 so every matmul runs in full
    128x128 array mode (partition sizes <= 64 drop into the much slower
    32x32-tile addressing mode).
  - C1 (scores, transposed): per chunk, PE matmul
        scoresT[keys, 128q] = k_aug_chunk.T @ q_aug_tile
    contracting over 65 channels: 16 data + 1 image-boundary indicator +
    16 y-onehot (mod 16) + 32 x-onehot (mod 32).  Position channels add 0
    for in-window pairs and -1e9 for out-of-window pairs, folding the 7x7
    window mask into the matmul (exact: no aliasing within +-10 / +-18).
  - exp: one ACT pass per 4 tiles, PSUM->SBUF, scale=1/sqrt(16), out bf16.
  - C2: PE matmul out_u[128q, 17] += expT_chunk.T @ vt_chunk; vt column 16
    is an inside-image indicator whose output is the softmax denominator Z.
  - One DVE copy per 4 tiles moves out_u PSUM->SBUF; the divide by Z
    happens on the host.
"""

import functools
import numpy as np
import ml_dtypes

B = 2
HEADS = 4
DH = 16
H = W = 96
PAD = 3
PH = PW = H + 2 * PAD  # 102
TY, TX = 8, 16           # query tile shape
GY, GX = H // TY, W // TX  # 12 x 6 tile grid
NT = GY * GX             # 72 tiles
HY, HX = TY + 6, TX + 6  # halo 14 x 22 = 308 keys
NK = HY * HX
NCH = 1 + 16 + 32        # img + y-onehot(16) + x-onehot(32)
D = DH + NCH             # 65 contraction channels
NEG = -1.0e9
CHUNKS = [(0, 128), (128, 115), (243, 65)]  # (key0, nkeys), all >= 65
NC = len(CHUNKS)
GB = 4                   # tiles per exp/PSUM group
NG = NT // GB            # 18 groups
LGRP = 12                # tiles per store DMA slice

BF16 = ml_dtypes.bfloat16

Y_OK = {d % 16 for d in range(-3, 4)}
X_OK = {d % 32 for d in range(-3, 4)}


@functools.lru_cache(maxsize=1)
def _mask_channels():
    """Constant (core-independent) aug channels.

    q side: [NCH, H, W]  (ones, y-onehot, x-onehot)
    k side: [NCH, PH, PW] (img indicator, y/x veto maps): 0 where the
    channel does not veto the pair, NEG where it does.
    """
    qm = np.zeros((NCH, H, W), np.float32)
    qm[0] = 1.0
    yy = np.arange(H)
    xx = np.arange(W)
    for r in range(16):
        qm[1 + r, yy % 16 == r, :] = 1.0
    for r in range(32):
        qm[17 + r, :, xx % 32 == r] = 1.0

    km = np.zeros((NCH, PH, PW), np.float32)
    km[0] = NEG
    km[0, PAD:PAD + H, PAD:PAD + W] = 0.0
    py = np.arange(PH)
    px = np.arange(PW)
    for r in range(16):
        bad = np.array([((y - PAD - r) % 16) not in Y_OK for y in py])
        km[1 + r, bad, :] = NEG
    for r in range(32):
        bad = np.array([((x - PAD - r) % 32) not in X_OK for x in px])
        km[17 + r, :, bad] = NEG
    return qm, km


def _host_prep(q, k, v):
    """Full [2,64,96,96] f32 inputs -> list of 8 per-core input dicts."""
    qm, km = _mask_channels()
    in_maps = []
    for core in range(8):
        b, h = divmod(core, HEADS)
        qs = q[b, DH * h:DH * h + DH]          # [16, 96, 96]
        ks = k[b, DH * h:DH * h + DH]
        vs = v[b, DH * h:DH * h + DH]

        q_aug = np.empty((D, H, W), np.float32)
        q_aug[:DH] = qs
        q_aug[DH:] = qm
        # [D, GY, TY, GX, TX] -> [D, GY*GX, TY*TX]
        q_tiled = np.ascontiguousarray(
            q_aug.reshape(D, GY, TY, GX, TX).transpose(0, 1, 3, 2, 4)
            .reshape(D, NT, TY * TX))

        k_aug = np.empty((D, PH, PW), np.float32)
        k_aug[:DH] = 0.0
        k_aug[:DH, PAD:PAD + H, PAD:PAD + W] = ks
        k_aug[DH:] = km
        # per-tile halo, flattened: [D, 72, 308]
        k_tiled = np.empty((D, NT, NK), np.float32)
        for ty in range(GY):
            for tx in range(GX):
                k_tiled[:, ty * GX + tx] = k_aug[
                    :, TY * ty:TY * ty + HY,
                    TX * tx:TX * tx + HX].reshape(D, NK)

        vt = np.zeros((PH, PW, DH + 1), np.float32)
        vt[PAD:PAD + H, PAD:PAD + W, :DH] = np.transpose(vs, (1, 2, 0))
        vt[PAD:PAD + H, PAD:PAD + W, DH] = 1.0
        # key-within-chunk major: [128, 72, 3, 17]
        vt_res = np.zeros((128, NT, NC, DH + 1), np.float32)
        for ty in range(GY):
            for tx in range(GX):
                halo = vt[TY * ty:TY * ty + HY,
                          TX * tx:TX * tx + HX, :].reshape(NK, DH + 1)
                for c, (k0, nk) in enumerate(CHUNKS):
                    vt_res[:nk, ty * GX + tx, c, :] = halo[k0:k0 + nk]

        in_maps.append({
            "q_tiled": q_tiled.astype(BF16),
            "k_tiled": k_tiled.astype(BF16),
            "vt_res": vt_res.astype(BF16),
        })
    return in_maps


@functools.lru_cache(maxsize=1)
def _build_program():
    from contextlib import ExitStack
    import concourse.bass as bass
    import concourse.tile as tile
    from concourse import bacc, mybir

    f32 = mybir.dt.float32
    bf16 = mybir.dt.bfloat16

    nc = bacc.Bacc("TRN2", target_bir_lowering=False, debug=False,
                   num_devices=8)
    q_d = nc.dram_tensor("q_tiled", [D, NT, TY * TX], bf16,
                         kind="ExternalInput").ap()
    k_d = nc.dram_tensor("k_tiled", [D, NT, NK], bf16,
                         kind="ExternalInput").ap()
    vt_d = nc.dram_tensor("vt_res", [128, NT, NC, DH + 1], bf16,
                          kind="ExternalInput").ap()
    # un-normalized [query-in-tile, tile, 17]; host divides by column 16
    out_d = nc.dram_tensor("out", [TY * TX, NT, DH + 1], f32,
                           kind="ExternalOutput").ap()

    with tile.TileContext(nc) as tc:
        with ExitStack() as ctx:
            cpool = ctx.enter_context(tc.tile_pool(name="const", bufs=1))
            epool = ctx.enter_context(tc.tile_pool(name="expT", bufs=3))
            pp_s = ctx.enter_context(
                tc.tile_pool(name="ps_scores", bufs=2, space="PSUM"))
            pp_o = ctx.enter_context(
                tc.tile_pool(name="ps_out", bufs=2, space="PSUM"))

            q_sb = cpool.tile([D, NT, TY * TX], bf16)
            k_sb = cpool.tile([D, NT, NK], bf16)
            vt_sb = cpool.tile([128, NT, NC, DH + 1], bf16)
            out_sb = cpool.tile([TY * TX, NT, DH + 1], f32)
            # uneven slices: a small first slice unblocks compute early,
            # bigger later slices keep the descriptor-generation count low
            t0 = 0
            for n in (8, 16, 24, 24):
                s = slice(t0, t0 + n)
                nc.sync.dma_start(k_sb[:, s, :], k_d[:, s, :])
                nc.sync.dma_start(q_sb[:, s, :], q_d[:, s, :])
                nc.sync.dma_start(vt_sb[:, s, :, :], vt_d[:, s, :, :])
                t0 += n

            exps = [None] * NG

            def emit_c1(g):
                scores = pp_s.tile([128, 3 * GB, 128], f32, tag="scores")
                for i in range(GB):
                    t = GB * g + i
                    for c, (k0, nk) in enumerate(CHUNKS):
                        nc.tensor.matmul(
                            scores[:nk, 3 * i + c, :],
                            lhsT=k_sb[:, t, k0:k0 + nk],
                            rhs=q_sb[:, t, :],
                            start=True, stop=True)
                expT = epool.tile([128, 3 * GB, 128], bf16, tag="expT")
                nc.scalar.activation(expT[:], scores[:],
                                     mybir.ActivationFunctionType.Exp,
                                     scale=0.25)
                exps[g] = expT

            def emit_c2(g):
                expT = exps[g]
                out_u = pp_o.tile([128, GB, 128], f32, tag="outu")
                for i in range(GB):
                    t = GB * g + i
                    for c, (k0, nk) in enumerate(CHUNKS):
                        nc.tensor.matmul(
                            out_u[:, i, :DH + 1],
                            lhsT=expT[:nk, 3 * i + c, :],
                            rhs=vt_sb[:nk, t, c, :],
                            start=(c == 0), stop=(c == 2))
                nc.vector.tensor_copy(
                    out_sb[:, GB * g:GB * g + GB, :],
                    out_u[:, :, :DH + 1])
                # store as soon as a 12-tile span is done
                if (g + 1) % (LGRP // GB) == 0:
                    s = slice(GB * g + GB - LGRP, GB * g + GB)
                    nc.sync.dma_start(out_d[:, s, :], out_sb[:, s, :])

            # software pipeline: PE never waits on the exp it just issued
            for g in range(NG):
                emit_c1(g)
                if g >= 2:
                    emit_c2(g - 2)
            emit_c2(NG - 2)
            emit_c2(NG - 1)
    nc.compile()
    return nc


def kernel(q, k, v):
    from concourse.bass_utils import run_bass_kernel_spmd

    nc = _build_program()
    in_maps = _host_prep(np.asarray(q, np.float32), np.asarray(k, np.float32),
                         np.asarray(v, np.float32))
    res = run_bass_kernel_spmd(nc, in_maps, list(range(8)))

    out = np.empty((B, HEADS, DH, H, W), np.float32)
    for core in range(8):
        b, h = divmod(core, HEADS)
        o = res.results[core]["out"]           # [TY*TX, NT, 17]
        o = o[:, :, :DH] / o[:, :, DH:DH + 1]  # softmax denominator
        # [TY*TX, NT, DH] -> [qy,qx,ty,tx,d] -> [ty,qy,tx,qx,d] -> [H,W,DH]
        o = o.reshape(TY, TX, GY, GX, DH)
        o = o.transpose(2, 0, 3, 1, 4).reshape(H, W, DH)
        out[b, h] = np.transpose(o, (2, 0, 1))
    return out.reshape(B, HEADS * DH, H, W)


# revision 6
# speedup vs baseline: 1.4730x; 1.1712x over previous
"""ConvLocalAttention2d (7x7 window, 4 heads, d_head=16) on 8 trn2 NeuronCores.

Sharding: one (batch, head) pair per core  (B=2 x heads=4 = 8 cores), zero
cross-core communication.

Per-core algorithm (d=16, H=W=96, K=7x7=49):
  - Queries are processed in 8x16 = 128-pixel tiles (12 x 6 = 72 tiles),
    ordered phase-major (by (ty%2, tx%2)) so mask constants can be
    broadcast on-device.
  - For each tile, the key/value halo is 14x22 = 308 pixels, zero-padded to
    321 and split into key chunks of (128, 128, 65).  All chunk sizes and
    the 65-channel contraction stay >= 65 (partition sizes <= 64 fall into
    the much slower 32x32-tile PE addressing mode) and 128-column weight
    loads take the 2x fast-weight-load path.
  - C1 (scores, transposed): per chunk, PE matmul
        scoresT[keys, 128q] = k_aug_chunk.T @ q_aug_tile
    contracting over 65 channels: 16 data + 1 image-boundary indicator +
    16 y-onehot (mod 16) + 32 x-onehot (mod 32).  Position channels add 0
    for in-window pairs and -1e9 for out-of-window pairs, folding the 7x7
    window mask into the matmul (exact: no aliasing within +-10 / +-18).
    Only the 16 data channels come from HBM; the 49 mask channels have just
    4 distinct per-tile patterns, which a DVE broadcast-copy replicates
    into the merged SBUF tiles (HBM traffic drops ~2.5x).
  - exp: one ACT pass per 4 tiles, PSUM->SBUF, scale=1/sqrt(16), out bf16.
  - C2: PE matmul out_u[128q, 17] += expT_chunk.T @ vt_chunk; vt column 16
    is an inside-image indicator whose output is the softmax denominator Z.
  - One DVE copy per 4 tiles moves out_u PSUM->SBUF; the divide by Z
    happens on the host.
"""

import functools
import numpy as np
import ml_dtypes

B = 2
HEADS = 4
DH = 16
H = W = 96
PAD = 3
PH = PW = H + 2 * PAD  # 102
TY, TX = 8, 16           # query tile shape
GY, GX = H // TY, W // TX  # 12 x 6 tile grid
NT = GY * GX             # 72 tiles
HY, HX = TY + 6, TX + 6  # halo 14 x 22 = 308 keys
NK = HY * HX
NKP = 321                # halo padded so chunks are (128, 128, 65)
NCH = 1 + 16 + 32        # img + y-onehot(16) + x-onehot(32)
NM = NCH                 # 49 mask channels
D = DH + NCH             # 65 contraction channels
NEG = -1.0e9
CHUNKS = [(0, 128), (128, 128), (256, 65)]  # (key0, nkeys), all >= 65
NC = len(CHUNKS)
GB = 4                   # tiles per exp/PSUM group
NG = NT // GB            # 18 groups
LGRP = 12                # tiles per store DMA slice
NPH = 4                  # mask phases (ty%2, tx%2)
PHN = NT // NPH          # tiles per phase

BF16 = ml_dtypes.bfloat16

Y_OK = {d % 16 for d in range(-3, 4)}
X_OK = {d % 32 for d in range(-3, 4)}


def _phase_perm():
    """perm[u] = standard tile index for phase-major slot u."""
    perm = []
    for p in range(NPH):
        a, b = divmod(p, 2)
        for ty2 in range(GY // 2):
            for tx2 in range(GX // 2):
                perm.append((2 * ty2 + a) * GX + (2 * tx2 + b))
    return np.array(perm)


@functools.lru_cache(maxsize=1)
def _mask_channels():
    """Constant (core-independent) aug channels.

    qm: [49, H, W]  (ones, y-onehot, x-onehot)
    km: [49, PH, PW] (img indicator, y/x veto maps): 0 where the channel
    does not veto the pair, NEG where it does.
    """
    qm = np.zeros((NM, H, W), np.float32)
    qm[0] = 1.0
    yy = np.arange(H)
    xx = np.arange(W)
    for r in range(16):
        qm[1 + r, yy % 16 == r, :] = 1.0
    for r in range(32):
        qm[17 + r, :, xx % 32 == r] = 1.0

    # NOTE: no image-boundary channel (it is not phase-periodic).  It is
    # also redundant: out-of-image keys have k=0 (score 0, exp 1) and
    # all-zero vt rows, so they contribute nothing to out or Z.
    km = np.zeros((NM, PH, PW), np.float32)
    py = np.arange(PH)
    px = np.arange(PW)
    for r in range(16):
        bad = np.array([((y - PAD - r) % 16) not in Y_OK for y in py])
        km[1 + r, bad, :] = NEG
    for r in range(32):
        bad = np.array([((x - PAD - r) % 32) not in X_OK for x in px])
        km[17 + r, :, bad] = NEG

    # 4-phase per-tile patterns (tiles (ty, tx) with ty%2=a, tx%2=b share)
    km_pat = np.zeros((NM, NPH, NKP), np.float32)
    qm_pat = np.zeros((NM, NPH, TY * TX), np.float32)
    for p in range(NPH):
        a, b = divmod(p, 2)
        km_pat[:, p, :NK] = km[:, TY * a:TY * a + HY,
                               TX * b:TX * b + HX].reshape(NM, NK)
        qm_pat[:, p, :] = qm[:, TY * a:TY * a + TY,
                             TX * b:TX * b + TX].reshape(NM, TY * TX)
    return km_pat, qm_pat


def _host_prep(q, k, v):
    """Full [2,64,96,96] f32 inputs -> list of 8 per-core input dicts."""
    km_pat, qm_pat = _mask_channels()
    perm = _phase_perm()
    in_maps = []
    for core in range(8):
        b, h = divmod(core, HEADS)
        qs = q[b, DH * h:DH * h + DH]          # [16, 96, 96]
        ks = k[b, DH * h:DH * h + DH]
        vs = v[b, DH * h:DH * h + DH]

        # [16, GY, TY, GX, TX] -> [16, 72, 128], phase-major tile order
        q_tiled = np.ascontiguousarray(
            qs.reshape(DH, GY, TY, GX, TX).transpose(0, 1, 3, 2, 4)
            .reshape(DH, NT, TY * TX)[:, perm, :])

        kp = np.zeros((DH, PH, PW), np.float32)
        kp[:, PAD:PAD + H, PAD:PAD + W] = ks
        k_tiled = np.zeros((DH, NT, NKP), np.float32)
        for ty in range(GY):
            for tx in range(GX):
                k_tiled[:, ty * GX + tx, :NK] = kp[
                    :, TY * ty:TY * ty + HY,
                    TX * tx:TX * tx + HX].reshape(DH, NK)
        k_tiled = np.ascontiguousarray(k_tiled[:, perm, :])

        vt = np.zeros((PH, PW, DH + 1), np.float32)
        vt[PAD:PAD + H, PAD:PAD + W, :DH] = np.transpose(vs, (1, 2, 0))
        vt[PAD:PAD + H, PAD:PAD + W, DH] = 1.0
        # key-within-chunk major: [128, 72, 3, 17]
        vt_res = np.zeros((128, NT, NC, DH + 1), np.float32)
        halo_p = np.zeros((NKP, DH + 1), np.float32)
        for ty in range(GY):
            for tx in range(GX):
                halo_p[:NK] = vt[TY * ty:TY * ty + HY,
                                 TX * tx:TX * tx + HX, :].reshape(NK, DH + 1)
                for c, (k0, nk) in enumerate(CHUNKS):
                    vt_res[:nk, ty * GX + tx, c, :] = halo_p[k0:k0 + nk]
        vt_res = np.ascontiguousarray(vt_res[:, perm, :, :])

        in_maps.append({
            "q_data": q_tiled.astype(BF16),
            "k_data": k_tiled.astype(BF16),
            "vt_res": vt_res.astype(BF16),
            "km_pat": km_pat.astype(BF16),
            "qm_pat": qm_pat.astype(BF16),
        })
    return in_maps


@functools.lru_cache(maxsize=1)
def _build_program():
    from contextlib import ExitStack
    import concourse.bass as bass
    import concourse.tile as tile
    from concourse import bacc, mybir

    f32 = mybir.dt.float32
    bf16 = mybir.dt.bfloat16

    nc = bacc.Bacc("TRN2", target_bir_lowering=False, debug=False,
                   num_devices=8)
    q_d = nc.dram_tensor("q_data", [DH, NT, TY * TX], bf16,
                         kind="ExternalInput").ap()
    k_d = nc.dram_tensor("k_data", [DH, NT, NKP], bf16,
                         kind="ExternalInput").ap()
    vt_d = nc.dram_tensor("vt_res", [128, NT, NC, DH + 1], bf16,
                          kind="ExternalInput").ap()
    km_d = nc.dram_tensor("km_pat", [NM, NPH, NKP], bf16,
                          kind="ExternalInput").ap()
    qm_d = nc.dram_tensor("qm_pat", [NM, NPH, TY * TX], bf16,
                          kind="ExternalInput").ap()
    # un-normalized [query-in-tile, tile, 17]; host divides by column 16
    out_d = nc.dram_tensor("out", [TY * TX, NT, DH + 1], f32,
                           kind="ExternalOutput").ap()

    with tile.TileContext(nc) as tc:
        with ExitStack() as ctx:
            cpool = ctx.enter_context(tc.tile_pool(name="const", bufs=1))
            epool = ctx.enter_context(tc.tile_pool(name="expT", bufs=3))
            pp_s = ctx.enter_context(
                tc.tile_pool(name="ps_scores", bufs=2, space="PSUM"))
            pp_o = ctx.enter_context(
                tc.tile_pool(name="ps_out", bufs=2, space="PSUM"))

            # merged [data; mask] tiles the matmuls read from
            q_sb = cpool.tile([D, NT, TY * TX], bf16)
            k_sb = cpool.tile([D, NT, NKP], bf16)
            vt_sb = cpool.tile([128, NT, NC, DH + 1], bf16)
            out_sb = cpool.tile([TY * TX, NT, DH + 1], f32)
            km_sb = cpool.tile([NM, NPH, NKP], bf16)
            qm_sb = cpool.tile([NM, NPH, TY * TX], bf16)

            nc.sync.dma_start(km_sb[:], km_d)
            nc.sync.dma_start(qm_sb[:], qm_d)
            # replicate the 4 mask patterns into the merged tiles (SBUF-only
            # traffic; the DVE broadcast-copy does not touch HBM).  Mask
            # channels sit at partitions 0:49 so the DVE write starts at
            # partition 0 (engine partition-base must be 32-aligned).
            for p in range(NPH):
                s = slice(PHN * p, PHN * (p + 1))
                nc.vector.tensor_copy(
                    k_sb[:NM, s, :],
                    km_sb[:, p, :].unsqueeze(1).broadcast_to(
                        [NM, PHN, NKP]))
                nc.vector.tensor_copy(
                    q_sb[:NM, s, :],
                    qm_sb[:, p, :].unsqueeze(1).broadcast_to(
                        [NM, PHN, TY * TX]))
            # sliced data loads so early tiles unblock quickly
            t0 = 0
            for n in (8, 16, 48):
                s = slice(t0, t0 + n)
                nc.sync.dma_start(k_sb[NM:, s, :], k_d[:, s, :])
                nc.sync.dma_start(q_sb[NM:, s, :], q_d[:, s, :])
                nc.sync.dma_start(vt_sb[:, s, :, :], vt_d[:, s, :, :])
                t0 += n

            exps = [None] * NG

            def emit_c1(g):
                scores = pp_s.tile([128, 3 * GB, 128], f32, tag="scores")
                for i in range(GB):
                    t = GB * g + i
                    for c, (k0, nk) in enumerate(CHUNKS):
                        nc.tensor.matmul(
                            scores[:nk, 3 * i + c, :],
                            lhsT=k_sb[:, t, k0:k0 + nk],
                            rhs=q_sb[:, t, :],
                            start=True, stop=True)
                expT = epool.tile([128, 3 * GB, 128], bf16, tag="expT")
                nc.scalar.activation(expT[:], scores[:],
                                     mybir.ActivationFunctionType.Exp,
                                     scale=0.25)
                exps[g] = expT

            def emit_c2(g):
                expT = exps[g]
                out_u = pp_o.tile([128, GB, 128], f32, tag="outu")
                for i in range(GB):
                    t = GB * g + i
                    for c, (k0, nk) in enumerate(CHUNKS):
                        nc.tensor.matmul(
                            out_u[:, i, :DH + 1],
                            lhsT=expT[:nk, 3 * i + c, :],
                            rhs=vt_sb[:nk, t, c, :],
                            start=(c == 0), stop=(c == 2))
                nc.vector.tensor_copy(
                    out_sb[:, GB * g:GB * g + GB, :],
                    out_u[:, :, :DH + 1])
                # store as soon as a 12-tile span is done
                if (g + 1) % (LGRP // GB) == 0:
                    s = slice(GB * g + GB - LGRP, GB * g + GB)
                    nc.sync.dma_start(out_d[:, s, :], out_sb[:, s, :])

            # software pipeline: PE never waits on the exp it just issued
            for g in range(NG):
                emit_c1(g)
                if g >= 2:
                    emit_c2(g - 2)
            emit_c2(NG - 2)
            emit_c2(NG - 1)
    nc.compile()
    return nc


def kernel(q, k, v):
    from concourse.bass_utils import run_bass_kernel_spmd

    nc = _build_program()
    in_maps = _host_prep(np.asarray(q, np.float32), np.asarray(k, np.float32),
                         np.asarray(v, np.float32))
    res = run_bass_kernel_spmd(nc, in_maps, list(range(8)))

    perm = _phase_perm()
    inv = np.empty_like(perm)
    inv[perm] = np.arange(NT)
    out = np.empty((B, HEADS, DH, H, W), np.float32)
    for core in range(8):
        b, h = divmod(core, HEADS)
        o = res.results[core]["out"][:, inv, :]  # undo phase-major order
        o = o[:, :, :DH] / o[:, :, DH:DH + 1]    # softmax denominator
        # [TY*TX, NT, DH] -> [qy,qx,ty,tx,d] -> [ty,qy,tx,qx,d] -> [H,W,DH]
        o = o.reshape(TY, TX, GY, GX, DH)
        o = o.transpose(2, 0, 3, 1, 4).reshape(H, W, DH)
        out[b, h] = np.transpose(o, (2, 0, 1))
    return out.reshape(B, HEADS * DH, H, W)


# revision 8
# speedup vs baseline: 1.8258x; 1.2395x over previous
"""ConvLocalAttention2d (7x7 window, 4 heads, d_head=16) on 8 trn2 NeuronCores.

Sharding: one (batch, head) pair per core  (B=2 x heads=4 = 8 cores), zero
cross-core communication.

Per-core algorithm (d=16, H=W=96, K=7x7=49):
  - Queries are processed in 12x8 = 96-pixel tiles (8 x 12 = 96 tiles),
    ordered phase-major (by (ty%2, tx%2)) so mask constants can be
    broadcast on-device.  The 12x8 shape gives an 18x14 = 252-key halo that
    fits TWO 128-key chunks, which is what prices both the PE stream time
    (chunks x queries columns) and the exp free-size (chunks x queries).
  - C1 (scores, transposed): per chunk, PE matmul
        scoresT[128 keys, 96q] = k_aug_chunk.T @ q_aug_tile
    contracting over 65 channels: 16 data + 24 y-onehot (mod 24) +
    16 x-onehot (mod 16) + 9 zero pad (partition sizes <= 64 fall into the
    much slower 32x32-tile PE addressing mode, so stay >= 65).  Position
    channels add 0 for in-window pairs and -1e9 for out-of-window pairs,
    folding the 7x7 window mask into the matmul (exact: no aliasing within
    dy in [-14,14], dx in [-10,10]).  Only the 16 data channels come from
    HBM; the mask channels have 4 per-tile patterns which a DVE
    broadcast-copy replicates into the merged SBUF tiles.
  - exp: one ACT pass per 6 tiles, PSUM->SBUF, scale=1/sqrt(16), out bf16,
    written into [*, *, :96] of a 128-wide expT tile so C2's weight loads
    stay 128-column (fast-weight-load path); the 96:128 junk columns only
    feed unread output partitions.
  - C2: PE matmul out_u[128q(96 valid), 17] += expT_chunk.T @ vt_chunk;
    vt column 16 is an inside-image indicator whose output is the softmax
    denominator Z.  (No image-boundary mask channel is needed: it is not
    phase-periodic, and out-of-image keys have k=0 -> score 0 -> exp 1
    with all-zero vt rows, contributing nothing to out or Z.)
  - One DVE copy per 6 tiles moves out_u PSUM->SBUF; the divide by Z
    happens on the host.
"""

import functools
import numpy as np
import ml_dtypes

B = 2
HEADS = 4
DH = 16
H = W = 96
PAD = 3
PH = PW = H + 2 * PAD  # 102
TY, TX = 12, 8           # query tile shape
GY, GX = H // TY, W // TX  # 8 x 12 tile grid
NT = GY * GX             # 96 tiles
NQ = TY * TX             # 96 queries per tile
HY, HX = TY + 6, TX + 6  # halo 18 x 14 = 252 keys
NK = HY * HX
NKP = 256                # halo padded so chunks are (128, 128)
MY, MX = 24, 16          # mask moduli (alias-free for |dy|<=14, |dx|<=10)
NM = MY + MX + 9         # mask channels (incl. 9 zero pads) = 49
D = DH + NM              # 65 contraction channels
NEG = -1.0e9
CHUNKS = [(0, 128), (128, 128)]
NC = len(CHUNKS)
GB = 6                   # tiles per exp/PSUM group
NG = NT // GB            # 16 groups
LGRP = 12                # tiles per store DMA slice
NPH = 4                  # mask phases (ty%2, tx%2)
PHN = NT // NPH          # tiles per phase = 24

BF16 = ml_dtypes.bfloat16

Y_OK = {d % MY for d in range(-3, 4)}
X_OK = {d % MX for d in range(-3, 4)}


def _phase_perm():
    """perm[u] = standard tile index for phase-major slot u."""
    perm = []
    for p in range(NPH):
        a, b = divmod(p, 2)
        for ty2 in range(GY // 2):
            for tx2 in range(GX // 2):
                perm.append((2 * ty2 + a) * GX + (2 * tx2 + b))
    return np.array(perm)


@functools.lru_cache(maxsize=1)
def _mask_channels():
    """Constant (core-independent) aug channels as 4-phase patterns.

    qm: one-hots of (y mod 24) then (x mod 16), then zero pads.
    km: 0 where the channel does not veto the pair, NEG where it does.
    """
    qm = np.zeros((NM, H, W), np.float32)
    yy = np.arange(H)
    xx = np.arange(W)
    for r in range(MY):
        qm[r, yy % MY == r, :] = 1.0
    for r in range(MX):
        qm[MY + r, :, xx % MX == r] = 1.0

    km = np.zeros((NM, PH, PW), np.float32)
    py = np.arange(PH)
    px = np.arange(PW)
    for r in range(MY):
        bad = np.array([((y - PAD - r) % MY) not in Y_OK for y in py])
        km[r, bad, :] = NEG
    for r in range(MX):
        bad = np.array([((x - PAD - r) % MX) not in X_OK for x in px])
        km[MY + r, :, bad] = NEG

    # 4-phase per-tile patterns (tiles (ty, tx) with ty%2=a, tx%2=b share)
    km_pat = np.zeros((NM, NPH, NKP), np.float32)
    qm_pat = np.zeros((NM, NPH, NQ), np.float32)
    for p in range(NPH):
        a, b = divmod(p, 2)
        km_pat[:, p, :NK] = km[:, TY * a:TY * a + HY,
                               TX * b:TX * b + HX].reshape(NM, NK)
        qm_pat[:, p, :] = qm[:, TY * a:TY * a + TY,
                             TX * b:TX * b + TX].reshape(NM, NQ)
    return km_pat, qm_pat


def _host_prep(q, k, v):
    """Full [2,64,96,96] f32 inputs -> list of 8 per-core input dicts."""
    km_pat, qm_pat = _mask_channels()
    perm = _phase_perm()
    in_maps = []
    for core in range(8):
        b, h = divmod(core, HEADS)
        qs = q[b, DH * h:DH * h + DH]          # [16, 96, 96]
        ks = k[b, DH * h:DH * h + DH]
        vs = v[b, DH * h:DH * h + DH]

        # [16, GY, TY, GX, TX] -> [16, 96, 96], phase-major tile order
        q_tiled = np.ascontiguousarray(
            qs.reshape(DH, GY, TY, GX, TX).transpose(0, 1, 3, 2, 4)
            .reshape(DH, NT, NQ)[:, perm, :])

        kp = np.zeros((DH, PH, PW), np.float32)
        kp[:, PAD:PAD + H, PAD:PAD + W] = ks
        k_tiled = np.zeros((DH, NT, NKP), np.float32)
        for ty in range(GY):
            for tx in range(GX):
                k_tiled[:, ty * GX + tx, :NK] = kp[
                    :, TY * ty:TY * ty + HY,
                    TX * tx:TX * tx + HX].reshape(DH, NK)
        k_tiled = np.ascontiguousarray(k_tiled[:, perm, :])

        vt = np.zeros((PH, PW, DH + 1), np.float32)
        vt[PAD:PAD + H, PAD:PAD + W, :DH] = np.transpose(vs, (1, 2, 0))
        vt[PAD:PAD + H, PAD:PAD + W, DH] = 1.0
        # key-within-chunk major: [128, 96, 2, 17]
        vt_res = np.zeros((128, NT, NC, DH + 1), np.float32)
        halo_p = np.zeros((NKP, DH + 1), np.float32)
        for ty in range(GY):
            for tx in range(GX):
                halo_p[:NK] = vt[TY * ty:TY * ty + HY,
                                 TX * tx:TX * tx + HX, :].reshape(NK, DH + 1)
                for c, (k0, nk) in enumerate(CHUNKS):
                    vt_res[:nk, ty * GX + tx, c, :] = halo_p[k0:k0 + nk]
        vt_res = np.ascontiguousarray(vt_res[:, perm, :, :])

        in_maps.append({
            "q_data": q_tiled.astype(BF16),
            "k_data": k_tiled.astype(BF16),
            "vt_res": vt_res.astype(BF16),
            "km_pat": km_pat.astype(BF16),
            "qm_pat": qm_pat.astype(BF16),
        })
    return in_maps


@functools.lru_cache(maxsize=1)
def _build_program():
    from contextlib import ExitStack
    import concourse.bass as bass
    import concourse.tile as tile
    from concourse import bacc, mybir

    f32 = mybir.dt.float32
    bf16 = mybir.dt.bfloat16

    nc = bacc.Bacc("TRN2", target_bir_lowering=False, debug=False,
                   num_devices=8)
    q_d = nc.dram_tensor("q_data", [DH, NT, NQ], bf16,
                         kind="ExternalInput").ap()
    k_d = nc.dram_tensor("k_data", [DH, NT, NKP], bf16,
                         kind="ExternalInput").ap()
    vt_d = nc.dram_tensor("vt_res", [128, NT, NC, DH + 1], bf16,
                          kind="ExternalInput").ap()
    km_d = nc.dram_tensor("km_pat", [NM, NPH, NKP], bf16,
                          kind="ExternalInput").ap()
    qm_d = nc.dram_tensor("qm_pat", [NM, NPH, NQ], bf16,
                          kind="ExternalInput").ap()
    # un-normalized [query-in-tile, tile, 17]; host divides by column 16
    out_d = nc.dram_tensor("out", [NQ, NT, DH + 1], f32,
                           kind="ExternalOutput").ap()

    with tile.TileContext(nc) as tc:
        with ExitStack() as ctx:
            cpool = ctx.enter_context(tc.tile_pool(name="const", bufs=1))
            epool = ctx.enter_context(tc.tile_pool(name="expT", bufs=3))
            pp_s = ctx.enter_context(
                tc.tile_pool(name="ps_scores", bufs=2, space="PSUM"))
            pp_o = ctx.enter_context(
                tc.tile_pool(name="ps_out", bufs=2, space="PSUM"))

            # merged [mask; data] tiles the matmuls read from (mask channels
            # first: DVE writes must start at a 32-aligned partition)
            q_sb = cpool.tile([D, NT, NQ], bf16)
            k_sb = cpool.tile([D, NT, NKP], bf16)
            vt_sb = cpool.tile([128, NT, NC, DH + 1], bf16)
            out_sb = cpool.tile([NQ, NT, DH + 1], f32)
            km_sb = cpool.tile([NM, NPH, NKP], bf16)
            qm_sb = cpool.tile([NM, NPH, NQ], bf16)

            nc.sync.dma_start(km_sb[:], km_d)
            nc.sync.dma_start(qm_sb[:], qm_d)
            # replicate the 4 mask patterns into the merged tiles (SBUF-only
            # traffic; the DVE broadcast-copy does not touch HBM).  Phase 0
            # is split so the first compute group unblocks early.
            spans = [(0, 0, 6), (0, 6, PHN)] + [
                (p, 0, PHN) for p in range(1, NPH)]
            for p, j0, j1 in spans:
                s = slice(PHN * p + j0, PHN * p + j1)
                nc.vector.tensor_copy(
                    k_sb[:NM, s, :],
                    km_sb[:, p, :].unsqueeze(1).broadcast_to(
                        [NM, j1 - j0, NKP]))
                nc.vector.tensor_copy(
                    q_sb[:NM, s, :],
                    qm_sb[:, p, :].unsqueeze(1).broadcast_to(
                        [NM, j1 - j0, NQ]))
            # sliced data loads so early tiles unblock quickly
            t0 = 0
            for n in (12, 24, 60):
                s = slice(t0, t0 + n)
                nc.sync.dma_start(k_sb[NM:, s, :], k_d[:, s, :])
                nc.sync.dma_start(q_sb[NM:, s, :], q_d[:, s, :])
                nc.sync.dma_start(vt_sb[:, s, :, :], vt_d[:, s, :, :])
                t0 += n

            exps = [None] * NG

            def emit_c1(g):
                scores = pp_s.tile([128, NC * GB, 128], f32, tag="scores")
                for i in range(GB):
                    t = GB * g + i
                    for c, (k0, nk) in enumerate(CHUNKS):
                        nc.tensor.matmul(
                            scores[:nk, NC * i + c, :NQ],
                            lhsT=k_sb[:, t, k0:k0 + nk],
                            rhs=q_sb[:, t, :],
                            start=True, stop=True)
                expT = epool.tile([128, NC * GB, 128], bf16, tag="expT")
                nc.scalar.activation(expT[:, :, :NQ], scores[:, :, :NQ],
                                     mybir.ActivationFunctionType.Exp,
                                     scale=0.25)
                exps[g] = expT

            def emit_c2(g):
                expT = exps[g]
                out_u = pp_o.tile([128, GB, 32], f32, tag="outu")
                for i in range(GB):
                    t = GB * g + i
                    for c, (k0, nk) in enumerate(CHUNKS):
                        nc.tensor.matmul(
                            out_u[:, i, :DH + 1],
                            lhsT=expT[:nk, NC * i + c, :],
                            rhs=vt_sb[:nk, t, c, :],
                            start=(c == 0), stop=(c == NC - 1))
                nc.vector.tensor_copy(
                    out_sb[:, GB * g:GB * g + GB, :],
                    out_u[:NQ, :, :DH + 1])
                # store as soon as a 12-tile span is done
                if (g + 1) % (LGRP // GB) == 0:
                    s = slice(GB * g + GB - LGRP, GB * g + GB)
                    nc.sync.dma_start(out_d[:, s, :], out_sb[:, s, :])

            # software pipeline: PE never waits on the exp it just issued
            for g in range(NG):
                emit_c1(g)
                if g >= 2:
                    emit_c2(g - 2)
            emit_c2(NG - 2)
            emit_c2(NG - 1)
    nc.compile()
    return nc


def kernel(q, k, v):
    from concourse.bass_utils import run_bass_kernel_spmd

    nc = _build_program()
    in_maps = _host_prep(np.asarray(q, np.float32), np.asarray(k, np.float32),
                         np.asarray(v, np.float32))
    res = run_bass_kernel_spmd(nc, in_maps, list(range(8)))

    perm = _phase_perm()
    inv = np.empty_like(perm)
    inv[perm] = np.arange(NT)
    out = np.empty((B, HEADS, DH, H, W), np.float32)
    for core in range(8):
        b, h = divmod(core, HEADS)
        o = res.results[core]["out"][:, inv, :]  # undo phase-major order
        o = o[:, :, :DH] / o[:, :, DH:DH + 1]    # softmax denominator
        # [NQ, NT, DH] -> [qy,qx,ty,tx,d] -> [ty,qy,tx,qx,d] -> [H,W,DH]
        o = o.reshape(TY, TX, GY, GX, DH)
        o = o.transpose(2, 0, 3, 1, 4).reshape(H, W, DH)
        out[b, h] = np.transpose(o, (2, 0, 1))
    return out.reshape(B, HEADS * DH, H, W)
